# revision 18
# baseline (speedup 1.0000x reference)
"""AttentionFusion kernel for 8 TRN2 NeuronCores.

Reference computation:
    expanded_video = repeat_interleave(video, 20, dim=1)        # [B, 1280, D]
    scores = expanded_video @ text.T * D**-0.5                  # [B, 1280, 256]
    attn_out = softmax(scores) @ text                           # [B, 1280, D]
    out = concat([text, video, expanded_video + attn_out], 1)   # [B, 1600, D]

Key algebraic fact: repeated (identical) query rows produce identical
attention outputs, so only the 64 unique video rows per batch need
attention; the 20x replication happens on the host during unsharding.

Sharding (v8): one core PAIR per batch, split along the SOFTMAX K
dimension (256 text rows -> 128 per core), zero cross-core traffic.
Each core computes attention over its own k-half, normalized by its
LOCAL row-sum s_h:
    E_h = exp(scores_h * scale);  A_h = (E_h / s_h) @ T_h
and the host combines:  attn = (s_0*A_0 + s_1*A_1) / (s_0 + s_1).

Dtype/engine choices (all trace-derived):
- stage 1 fp8e4 DoubleRow (2 contraction rows/partition): 40 matmuls
  at ~107 ns, finishes inside the qtt stream.
- stage 2 MIXED dtype: lhsT fp16 (normalized weights), rhs fp8e3
  values - hw-validated exact; halves the value-stream bytes vs fp16
  while keeping the fast dual-tile_position 512-col matmul form.
- output fp8e3 (normalized attn is O(1), e3m4 max 15.5 is plenty).
- DMA: per-engine queues are FIFO in trigger order; per-packet fixed
  cost ~80 ns, so packets are kept >= 4 KB where possible.  Input DMA
  count stays inside the DMA-semaphore pool (more DMAs than semaphores
  serializes trigger programming behind prior chunk completions).

Host pre-transposes inputs into the layouts the TensorEngine needs
(contraction dim on partitions), so every DMA is contiguous.
"""

import sys

import numpy as np

if "/opt/trn_rl_repo" not in sys.path:
    sys.path.insert(0, "/opt/trn_rl_repo")

import ml_dtypes

REPEAT = 20
D = 10240
SCALE = D ** (-0.5)
B, TT, TV = 4, 256, 64
KH = 128          # k-half: text rows per core
DJ = 40           # stage-1 contraction chunks (256 d each, DoubleRow)
NR = 10           # stage-2 rounds; each = 2 col groups x 512 cols
NCORES = 8

_compiled = None


def _build():
    import concourse.mybir as mybir
    import concourse.tile as tile
    from concourse import bacc
    from concourse.masks import make_identity

    f32 = mybir.dt.float32
    f16 = mybir.dt.float16
    fp8s1 = mybir.dt.float8e4
    fp8v = mybir.dt.float8e3

    nc = bacc.Bacc(
        "TRN2", target_bir_lowering=False, debug=False, num_devices=NCORES
    )
    qtt_h = nc.dram_tensor(
        "qtt", [128, DJ, 2, TV + KH], fp8s1, kind="ExternalInput"
    )
    tn_h = nc.dram_tensor("tn", [128, NR * 2, 512], fp8v, kind="ExternalInput")
    out_h = nc.dram_tensor("out", [128, NR, 512], fp8v, kind="ExternalOutput")
    ls_h = nc.dram_tensor("lsum", [TV, 1], f32, kind="ExternalOutput")

    QJ = (16, 16, 8)  # qtt chunk sizes in j (last chunk small: earlier done)
    TNC = (10, 10)    # tn chunk sizes in 512-col slices

    with tile.TileContext(nc) as tc:
        with (
            tc.tile_pool(name="qtp", bufs=len(QJ)) as qt_pool,
            tc.tile_pool(name="tnp", bufs=len(TNC)) as tn_pool,
            tc.tile_pool(name="smp", bufs=1) as sm_pool,
            tc.tile_pool(name="osp", bufs=2) as os_pool,
            tc.tile_pool(name="ps_1", bufs=1, space="PSUM") as ps_1_pool,
            tc.tile_pool(name="ps_w", bufs=1, space="PSUM") as ps_w_pool,
            tc.tile_pool(name="ps_o", bufs=4, space="PSUM") as ps_o_pool,
        ):
            ident = sm_pool.tile([TV, TV], f16, tag="ident")
            make_identity(nc, ident[:])

            # stage 1: S_h = Q @ T_h.T in fp8e4 DoubleRow (single
            # accumulation group; tile_position is not legal with DR)
            ps1 = ps_1_pool.tile([TV, KH], f32)
            j0 = 0
            for jc in QJ:
                qsb = qt_pool.tile([128, jc, 2, TV + KH], fp8s1)
                nc.sync.dma_start(qsb[:], qtt_h[:, j0 : j0 + jc])
                for j in range(jc):
                    jj = j0 + j
                    nc.tensor.matmul(
                        ps1[:],
                        lhsT=qsb[:, j, :, 0:TV],
                        rhs=qsb[:, j, :, TV : TV + KH],
                        start=(jj == 0),
                        stop=(jj == DJ - 1),
                        perf_mode=mybir.MatmulPerfMode.DoubleRow,
                    )
                j0 += jc

            # stage-2 values stream in while stage 1 runs (FIFO after qtt)
            tn_sb = []
            n0 = 0
            for tc_sz in TNC:
                t = tn_pool.tile([128, tc_sz, 512], fp8v)
                nc.sync.dma_start(t[:], tn_h[:, n0 : n0 + tc_sz, :])
                tn_sb.append((t, n0, tc_sz))
                n0 += tc_sz

            # softmax (local k-half, unnormalized sums kept for the host):
            # E = exp(S*scale), s = rowsum(E), W = E / s
            e_sb = sm_pool.tile([TV, KH], f16, tag="e")
            lsum = sm_pool.tile([TV, 1], f32, tag="lsum")
            nc.scalar.activation(
                e_sb[:],
                ps1[:],
                mybir.ActivationFunctionType.Exp,
                scale=SCALE,
                accum_out=lsum[:],
            )
            nc.gpsimd.dma_start(ls_h[:], lsum[:])
            rl = sm_pool.tile([TV, 1], f32, tag="rl")
            nc.vector.reciprocal(rl[:], lsum[:])
            w_sb = sm_pool.tile([TV, KH], f16, tag="w")
            nc.vector.tensor_scalar_mul(w_sb[:], e_sb[:], rl[:])

            # W[64, 128] -> WT[128, 64] (k on partitions) via PE transpose
            wt_ps = ps_w_pool.tile([KH, TV], f16)
            nc.tensor.transpose(wt_ps[:], w_sb[:], ident[:])
            wt_sb = sm_pool.tile([KH, TV], f16, tag="wt")
            nc.scalar.copy(wt_sb[:], wt_ps[:])

            # stage 2: A_h = W @ T_h, 2x column-tiled (same fp16 weights at
            # two tile positions, two fp8e3 rhs streams)
            def rhs_for(n):
                for t, s0, sz in tn_sb:
                    if s0 <= n < s0 + sz:
                        return t[:, n - s0, :]
                raise AssertionError

            osb = None
            for r in range(NR):
                ps_o = ps_o_pool.tile([128, 512], f32)
                for g in range(2):
                    nc.tensor.matmul(
                        ps_o[g * TV : (g + 1) * TV, :],
                        lhsT=wt_sb[:],
                        rhs=rhs_for(2 * r + g),
                        start=True,
                        stop=True,
                        tile_position=(0, g * TV),
                        skip_group_check=True,
                    )
                if r % 5 == 0:
                    osb = os_pool.tile([128, 5, 512], fp8v)
                if r % 2 == 0:
                    nc.vector.tensor_copy(osb[:, r % 5, :], ps_o[:])
                else:
                    nc.scalar.copy(osb[:, r % 5, :], ps_o[:])
                if r % 5 == 4:
                    # batched 2560 B/partition write at full HWDGE rate;
                    # sync/gpsimd are both idle here
                    eng = nc.sync if r == 4 else nc.gpsimd
                    eng.dma_start(out_h[:, r - 4 : r + 1, :], osb[:])

    nc.compile()
    return nc


def _prepare_in_maps(text, video):
    tf = np.asarray(text, dtype=np.float32)
    vf = np.asarray(video, dtype=np.float32)
    t8 = tf.astype(ml_dtypes.float8_e4m3)
    v8 = vf.astype(ml_dtypes.float8_e4m3)
    t8v = tf.astype(ml_dtypes.float8_e3m4)
    in_maps = []
    for c in range(NCORES):
        b, h = divmod(c, 2)
        # qtt[p, j, i, q]      = video[b, q, j*256 + i*128 + p]
        # qtt[p, j, i, 64+kk]  = text[b, h*128 + kk, j*256 + i*128 + p]
        qtt = np.empty((128, DJ, 2, TV + KH), dtype=ml_dtypes.float8_e4m3)
        qtt[:, :, :, 0:TV] = (
            v8[b].reshape(TV, DJ, 2, 128).transpose(3, 1, 2, 0)
        )
        qtt[:, :, :, TV:] = (
            t8[b, h * KH : (h + 1) * KH]
            .reshape(KH, DJ, 2, 128)
            .transpose(3, 1, 2, 0)
        )
        # tn[p, n, c] = text[b, h*128 + p, n*512 + c]  (fp8e3 values)
        tn = np.ascontiguousarray(
            t8v[b, h * KH : (h + 1) * KH].reshape(128, NR * 2, 512)
        )
        in_maps.append({"qtt": qtt, "tn": tn})
    return in_maps


def _assemble(results, text, video):
    tf = np.asarray(text, dtype=np.float32)
    vf = np.asarray(video, dtype=np.float32)
    onum = np.zeros((B, TV, D), np.float32)
    oden = np.zeros((B, TV, 1), np.float32)
    for c in range(NCORES):
        b, h = divmod(c, 2)
        a = np.asarray(results[c]["out"], dtype=np.float32)
        # out[64*g + q, r, x] = A_h[q, (2r+g)*512 + x]
        a = a.reshape(2, TV, NR, 512).transpose(1, 2, 0, 3).reshape(TV, D)
        s = np.asarray(results[c]["lsum"], dtype=np.float32).reshape(TV, 1)
        onum[b] += s * a
        oden[b] += s
    fused = vf + onum / oden
    return np.concatenate([tf, vf, np.repeat(fused, REPEAT, axis=1)], axis=1)


def _ensure_ntff_hook():
    """Register the axon NTFF profiling hook if the image lacks
    antenv.axon_hooks (trace=True degrades to no-op otherwise)."""
    import types

    try:
        from antenv import axon_hooks  # noqa: F401

        return
    except ImportError:
        pass
    mod = types.ModuleType("antenv.axon_hooks")
    _hook = [None]
    mod.set_axon_ntff_profile_hook = lambda h: _hook.__setitem__(0, h)
    mod.get_axon_ntff_profile_hook = lambda: _hook[0]
    sys.modules["antenv.axon_hooks"] = mod
    import antenv

    antenv.axon_hooks = mod
    try:
        from trn_agent_boot.trn_boot import _ntff_profile_via_ctypes

        mod.set_axon_ntff_profile_hook(
            _ntff_profile_via_ctypes("/opt/axon/libaxon_pjrt.so")
        )
    except Exception:
        pass


def _run(text_features, video_features, trace=False, **spmd_kwargs):
    global _compiled
    if _compiled is None:
        _compiled = _build()
    if trace:
        _ensure_ntff_hook()
    from concourse.bass_utils import run_bass_kernel_spmd

    in_maps = _prepare_in_maps(text_features, video_features)
    res = run_bass_kernel_spmd(
        _compiled,
        in_maps,
        core_ids=list(range(NCORES)),
        trace=trace,
        **spmd_kwargs,
    )
    out = _assemble(res.results, text_features, video_features)
    return out, res


def kernel(text_features, video_features):
    out, _ = _run(text_features, video_features)
    return out
